# revision 8
# baseline (speedup 1.0000x reference)
"""Cross-attention (GQA, key-padding + shifted-causal mask) on 8 Trainium2 cores.

Sharding: core k handles kv head k for BOTH batches (4 query heads each under
GQA) -> 8 (b,h) attention instances per core, no collectives. This balances
work across cores because per-batch work depends on the ragged length.

Mask algebra: the reference adds -10000 for padded keys and replaces with
-10000 where s > t + len_b - Sk. With c_b = Sk - len_b the effective rule is
"key s visible to query t iff s <= t - c_b" (causality subsumes padding since
t - c_b <= len_b - 1 always). So per query chunk only the PREFIX of s-blocks
up to the causal diagonal participates; c_b is read from the runtime mask and
the program is compiled per (c_0, c_1) (cached). Queries t < c_b attend to
nothing; the reference gives them a uniform softmax -> mean(V), patched on
host.

Per (b,h), in score-transposed layout [s, t] with TQ=256 query chunks:
  ST = K^T Q           (bf16 matmuls, one per 128-wide s block)
  P  = exp(scale*ST)   (ScalarE, grouped up to 6 s-blocks per call, fp16 out)
  P *= diag_mask       (gpsimd affine_select on partially-masked blocks only)
  OT   += V'[s,d] P[s,t]     (fp16 matmuls, PSUM accum over s blocks)
  Pacc2 += [P_even | P_odd]  (VectorE fp16 pairwise accumulate, [TS, 2*TQ])
  denT[m] = sum_p Pacc2[p, m]  (4 tiny [128,1]-out matmuls vs all-ones rhs,
                                accumulated into spare columns of the OT bank)
  [OT | denT] -> SBUF -> DRAM; host computes OT/den and transposes to
  (B, Sq, H, D).
"""

import numpy as np

B, SQ, SK, H, HK, D = 2, 2048, 2048, 32, 8, 128
G = H // HK            # query heads per kv head
N_CORES = 8
TQ = 256               # t (query) tile width
TS = 128               # s (key) tile width
NTQ = SQ // TQ         # 8 t-chunks
GRP = 4                # s-blocks per exp group / ST psum tile
SCALE = 1.0 / float(np.sqrt(D))

_compiled = {}


def _nb_table(c):
    """Number of s-blocks per t-chunk for shift c (prefix up to causal diag)."""
    nbmax = (SK - 1 - c) // TS + 1
    out = []
    for tc in range(NTQ):
        nb = (TQ * tc + TQ - 1 - c) // TS + 1
        out.append(min(max(nb, 0), nbmax))
    return out


def _build_program(c):
    """Build + schedule the SPMD Bass program, specialized on (c0, c1)."""
    from contextlib import ExitStack
    import concourse.bass as bass
    import concourse.tile as tile
    from concourse import bacc, mybir

    f32 = mybir.dt.float32
    bf16 = mybir.dt.bfloat16
    f16 = mybir.dt.float16

    nb_tabs = [_nb_table(ci) for ci in c]
    nbmaxs = [(SK - 1 - ci) // TS + 1 for ci in c]
    NBK = max(nbmaxs)

    nc = bacc.Bacc("TRN2", target_bir_lowering=False, debug=False)
    qT_ap = nc.dram_tensor("qT", [2 * G, D, SQ], bf16, kind="ExternalInput").ap()
    kT_ap = nc.dram_tensor("kT", [2, D, NBK * TS], bf16, kind="ExternalInput").ap()
    v_ap = nc.dram_tensor("v", [2, TS, NBK * D], f16, kind="ExternalInput").ap()
    out_ap = nc.dram_tensor("out", [2 * G, D, SQ], f32, kind="ExternalOutput").ap()
    den_ap = nc.dram_tensor("den", [2 * G, NTQ, TS, 4], f32,
                            kind="ExternalOutput").ap()

    with tile.TileContext(nc) as tc, ExitStack() as ctx:
        const_pool = ctx.enter_context(tc.tile_pool(name="const", bufs=1))
        kv_pool = ctx.enter_context(tc.tile_pool(name="kv", bufs=2))
        q_pool = ctx.enter_context(tc.tile_pool(name="q", bufs=2))
        p_pool = ctx.enter_context(tc.tile_pool(name="p", bufs=4))
        pacc_pool = ctx.enter_context(tc.tile_pool(name="pacc", bufs=2))
        osb_pool = ctx.enter_context(tc.tile_pool(name="osb", bufs=3))
        st_psum = ctx.enter_context(tc.tile_pool(name="st", bufs=3, space="PSUM"))
        ot_psum = ctx.enter_context(tc.tile_pool(name="ot", bufs=2, space="PSUM"))

        ones_sb = const_pool.tile([TS, 1], f16)
        nc.vector.memset(ones_sb[:], 1.0)

        pending = None  # 1-deep SW pipeline keeps PE ahead of ACT

        def flush(pend):
            # PV matmuls for a finished group; on the chunk's last group also
            # emit denT, the PSUM->SBUF copy and the output DMAs.
            for u in range(pend["gn"]):
                sc = pend["g0"] + u
                nc.tensor.matmul(
                    pend["ot_ps"][:, :TQ],
                    lhsT=pend["v_sb"][:, sc * D : (sc + 1) * D],
                    rhs=pend["p_sb"][:, u * TQ : (u + 1) * TQ],
                    start=(pend["first"] and u == 0),
                    stop=(pend["last"] and u == pend["gn"] - 1),
                )
            if pend["last"]:
                # denT into the spare columns of the OT bank: one single
                # start+stop matmul per column (sub-sums combined on host;
                # interleaved open accumulation groups corrupt PSUM).
                nk = 4 if pend["nb"] >= 2 else 2
                for k in range(nk):
                    nc.tensor.matmul(
                        pend["ot_ps"][:, TQ + k : TQ + k + 1],
                        lhsT=pend["pacc"][:, k * TS : (k + 1) * TS],
                        rhs=ones_sb[:, 0:1],
                        start=True,
                        stop=True,
                    )
                osb = osb_pool.tile([D, TQ + 4], f32)
                nc.vector.tensor_copy(
                    out=osb[:, : TQ + nk], in_=pend["ot_ps"][:, : TQ + nk])
                nc.sync.dma_start(
                    out_ap[pend["ih"]][:, pend["tc"] * TQ : (pend["tc"] + 1) * TQ],
                    osb[:, :TQ],
                )
                nc.sync.dma_start(
                    den_ap[pend["ih"]][pend["tc"]][:, :nk], osb[:, TQ : TQ + nk]
                )

        for i in range(2):  # batch
            cb = c[i]
            nbk = nbmaxs[i]
            nb_tab = nb_tabs[i]
            kT_sb = kv_pool.tile([D, NBK * TS], bf16, tag="kT")
            v_sb = kv_pool.tile([TS, NBK * D], f16, tag="v")
            if i == 0:
                # startup: make the first ST matmul's operands land early
                nc.sync.dma_start(kT_sb[:, :TS], kT_ap[i][:, :TS])
                nc.sync.dma_start(kT_sb[:, TS : nbk * TS],
                                  kT_ap[i][:, TS : nbk * TS])
            else:
                nc.sync.dma_start(kT_sb[:, : nbk * TS], kT_ap[i][:, : nbk * TS])
            nc.gpsimd.dma_start(v_sb[:, : nbk * D], v_ap[i][:, : nbk * D])

            for j in range(G):
                ih = i * G + j
                qT_sb = q_pool.tile([D, SQ], bf16)
                if i == 0 and j == 0:
                    nc.sync.dma_start(qT_sb[:, : 2 * TQ], qT_ap[ih][:, : 2 * TQ])
                    nc.sync.dma_start(qT_sb[:, 2 * TQ :], qT_ap[ih][:, 2 * TQ :])
                else:
                    nc.sync.dma_start(qT_sb[:], qT_ap[ih])

                for tcix in range(NTQ):
                    nb = nb_tab[tcix]
                    if nb == 0:
                        continue
                    ot_ps = ot_psum.tile([D, TQ + 4], f32)
                    pacc = pacc_pool.tile([TS, 2 * TQ], f16)
                    npair = 0  # pair-adds emitted so far this chunk
                    g0 = 0
                    while g0 < nb:
                        gn = min(GRP, nb - g0)
                        st_ps = st_psum.tile([TS, GRP * TQ], f32)
                        for u in range(gn):
                            sc = g0 + u
                            nc.tensor.matmul(
                                st_ps[:, u * TQ : (u + 1) * TQ],
                                lhsT=kT_sb[:, sc * TS : (sc + 1) * TS],
                                rhs=qT_sb[:, tcix * TQ : (tcix + 1) * TQ],
                                start=True,
                                stop=True,
                            )
                        p_sb = p_pool.tile([TS, GRP * TQ], f16)
                        nc.scalar.activation(
                            p_sb[:, : gn * TQ], st_ps[:, : gn * TQ],
                            mybir.ActivationFunctionType.Exp,
                            scale=SCALE,
                        )
                        for u in range(gn):
                            sc = g0 + u
                            bv = TS * sc + cb - TQ * tcix
                            if bv > -(TS - 1):  # partially masked block
                                nc.gpsimd.affine_select(
                                    out=p_sb[:, u * TQ : (u + 1) * TQ],
                                    in_=p_sb[:, u * TQ : (u + 1) * TQ],
                                    pattern=[[1, TQ]],
                                    compare_op=mybir.AluOpType.is_ge,
                                    fill=0.0,
                                    base=-bv,
                                    channel_multiplier=-1,
                                )
                        # pairwise accumulate: even blocks into pacc[:, :TQ],
                        # odd into pacc[:, TQ:]; remainder single into lo half
                        u = 0
                        while u < gn:
                            if u + 2 <= gn:
                                src = p_sb[:, u * TQ : (u + 2) * TQ]
                                if npair == 0:
                                    nc.vector.tensor_copy(out=pacc[:], in_=src)
                                else:
                                    nc.vector.tensor_tensor(
                                        out=pacc[:], in0=pacc[:], in1=src,
                                        op=mybir.AluOpType.add,
                                    )
                                npair += 1
                                u += 2
                            else:
                                src = p_sb[:, u * TQ : (u + 1) * TQ]
                                if npair == 0:  # nb == 1 chunk
                                    nc.vector.tensor_copy(
                                        out=pacc[:, :TQ], in_=src)
                                else:
                                    nc.vector.tensor_tensor(
                                        out=pacc[:, :TQ], in0=pacc[:, :TQ],
                                        in1=src, op=mybir.AluOpType.add,
                                    )
                                u += 1
                        if pending is not None:
                            flush(pending)
                        pending = {
                            "g0": g0, "gn": gn, "nb": nb, "p_sb": p_sb,
                            "v_sb": v_sb, "ot_ps": ot_ps, "pacc": pacc,
                            "ih": ih, "tc": tcix,
                            "first": g0 == 0, "last": g0 + gn >= nb,
                        }
                        g0 += gn

        if pending is not None:
            flush(pending)

    nc.compile()
    return nc


def _get_program(c):
    key = tuple(int(x) for x in c)
    if key not in _compiled:
        _compiled[key] = _build_program(key)
    return _compiled[key]


def kernel(q, kv, key_padding_mask, _want_trace=False):
    import ml_dtypes

    bf16 = ml_dtypes.bfloat16
    q = np.asarray(q, dtype=np.float32)
    kv = np.asarray(kv, dtype=np.float32)
    mask = np.asarray(key_padding_mask).astype(bool)

    lengths = mask.sum(axis=1).astype(np.int64)
    # contiguous-prefix masks assumed (reference builds them that way)
    assert all(mask[b, : lengths[b]].all() and not mask[b, lengths[b]:].any()
               for b in range(B))
    c = tuple(int(SK - l) for l in lengths)
    nbmaxs = [(SK - 1 - ci) // TS + 1 for ci in c]
    NBK = max(nbmaxs)

    k_full = kv[:, :, 0]  # (B, SK, HK, D)
    v_full = kv[:, :, 1]

    k_bf = k_full.astype(bf16)
    v_16 = v_full.astype(np.float16)
    q_bf = q.astype(bf16)

    in_maps = []
    for core in range(N_CORES):
        hk = core
        qT = np.empty((2 * G, D, SQ), dtype=bf16)
        kT = np.zeros((2, D, NBK * TS), dtype=bf16)
        v_l = np.zeros((2, TS, NBK * D), dtype=np.float16)
        for i in range(B):
            nbk = nbmaxs[i]
            kT[i, :, : nbk * TS] = k_bf[i, : nbk * TS, hk, :].T
            # v chunked: v_l[i][p, sc*D + d] = v[i, sc*TS + p, hk, d]
            v_l[i, :, : nbk * D] = np.ascontiguousarray(
                v_16[i, : nbk * TS, hk, :].reshape(nbk, TS, D).transpose(1, 0, 2)
            ).reshape(TS, nbk * D)
            for j in range(G):
                qT[i * G + j] = q_bf[i, :, hk * G + j, :].T
        in_maps.append({
            "qT": np.ascontiguousarray(qT),
            "kT": kT,
            "v": v_l,
        })

    from concourse.bass_utils import run_bass_kernel_spmd

    nc = _get_program(c)
    res = run_bass_kernel_spmd(
        nc, in_maps, core_ids=list(range(N_CORES)),
        trace=_want_trace,
    )

    out = np.empty((B, SQ, H, D), dtype=np.float32)
    for core in range(N_CORES):
        hk = core
        o_core = res.results[core]["out"]    # (2*G, D, SQ) f32
        den_core = res.results[core]["den"]  # (2*G, NTQ, TS, 4) f32
        for i in range(B):
            nb_tab = _nb_table(c[i])
            for j in range(G):
                ih = i * G + j
                # den[tc, p, k]: k in {0,1} lo-half sub-sums, {2,3} hi-half;
                # t = tc*TQ + (k&1)*TS + p. hi cols are garbage when nb==1.
                dc = den_core[ih].copy()
                for tc_i, nb_i in enumerate(nb_tab):
                    if nb_i == 1:
                        dc[tc_i, :, 2:] = 0.0
                den2 = dc[:, :, :2] + dc[:, :, 2:]
                den_t = den2.transpose(0, 2, 1).reshape(SQ)
                with np.errstate(divide="ignore", invalid="ignore"):
                    out[i, :, hk * G + j, :] = (o_core[ih] / den_t[None, :]).T

    # rows that attend to nothing: reference softmax is uniform -> mean(V)
    for b in range(B):
        if c[b] > 0:
            vm = v_full[b].mean(axis=0)  # (HK, D)
            out[b, : c[b]] = np.repeat(vm, G, axis=0)[None]

    if _want_trace:
        return out, res
    return out
